# revision 25
# baseline (speedup 1.0000x reference)
"""GIN-style GNN message passing on 8 TRN2 NeuronCores.

Pipeline (per core, nodes sharded by graph id so pooling is local):
  phase 1: edge aggregation  agg[dst] += x[src]
      - node tiles processed in chunks of CH_T=8; each chunk accumulates
        directly in PSUM across all 4 src blocks (no mid-phase evacuation).
      - edges bucketed by (chunk, src block b, node tile t), padded to
        groups of 128; one dma_gather call per (chunk, b) covers all its
        groups (bf16 256B rows); segment-sum via one-hot matmul into the
        per-tile PSUM accumulator.
  phase 2 (fused per chunk): hin = bf16(x + agg); h = relu(relu(hin @ w1
      + b1) @ w2 + b2) with bf16 weights; pooled per graph via one-hot
      matmul into a single PSUM accumulator; then the small MLP head +
      log_softmax in fp32.

The bass program is identical across the 8 cores (SPMD); all data-dependent
structure (bucket sizes) is made uniform by padding to the max over cores.
"""
import numpy as np
import ml_dtypes

import concourse.bacc as bacc
import concourse.tile as tile
from concourse import mybir
from concourse.bass_utils import run_bass_kernel_spmd
from concourse.library_config import mlp as mlp_lib

P = 128
F = 128
HID = 128
NCLS = 10
NCORES = 8
CH_T = 8      # node tiles per PSUM-accumulation chunk
BLK = 25088   # uniform src block size (int16 indices)
NBLK = 4
CALL_G = 8    # max groups per dma_gather call (ring-capacity limit)

FP32 = mybir.dt.float32
BF16 = mybir.dt.bfloat16
I16 = mybir.dt.int16


def _chunks(NT):
    return [list(range(c, min(NT, c + CH_T))) for c in range(0, NT, CH_T)]


def build_program(NT, G_tb, Np, blocks, GPG, rep=1):
    """Build the SPMD bass program.

    NT: node tiles per core; G_tb: [NT, NBLK] groups per bucket; Np: NT*P;
    blocks: src block sizes; GPG: graphs per core.
    """
    nblk = len(blocks)
    bstart = [0]
    for bs in blocks:
        bstart.append(bstart[-1] + bs)
    TOT_G = int(G_tb.sum())
    chunks = _chunks(NT)
    # max groups per gather call / per chunk (uniform tile shapes)
    GMAXC = max(int(G_tb[tiles, b].sum())
                for tiles in chunks for b in range(nblk))
    GMAXCH = max(int(G_tb[tiles, :].sum()) for tiles in chunks)

    nc = bacc.Bacc("TRN2", target_bir_lowering=False, debug=False,
                   num_swdge_queues=4)

    xb_t = nc.declare_dram_parameter("xb", [bstart[-1], F], BF16, isOutput=False)
    idx_t = nc.declare_dram_parameter("idx", [P, TOT_G * 8], I16, isOutput=False)
    dst_t = nc.declare_dram_parameter("dstc", [P, TOT_G], FP32, isOutput=False)
    xt_t = nc.declare_dram_parameter("xt", [P, Np], FP32, isOutput=False)
    bc_t = nc.declare_dram_parameter("bc", [P, NT], FP32, isOutput=False)
    w1_t = nc.declare_dram_parameter("w1", [F, HID], BF16, isOutput=False)
    b1_t = nc.declare_dram_parameter("b1", [HID, 1], FP32, isOutput=False)
    w2_t = nc.declare_dram_parameter("w2", [HID, HID], BF16, isOutput=False)
    b2_t = nc.declare_dram_parameter("b2", [HID, 1], FP32, isOutput=False)
    l1w_t = nc.declare_dram_parameter("l1w", [HID, HID], FP32, isOutput=False)
    l1b_t = nc.declare_dram_parameter("l1b", [HID, 1], FP32, isOutput=False)
    l2w_t = nc.declare_dram_parameter("l2w", [HID, NCLS], FP32, isOutput=False)
    l2b_t = nc.declare_dram_parameter("l2b", [GPG, NCLS], FP32, isOutput=False)
    out_t = nc.declare_dram_parameter("out", [GPG, NCLS], FP32, isOutput=True)

    iota_c = nc.inline_tensor(
        np.tile(np.arange(P, dtype=ml_dtypes.bfloat16), (P, 1)), name="iota128")
    iotag_c = nc.inline_tensor(
        np.tile(np.arange(GPG, dtype=ml_dtypes.bfloat16), (P, 1)), name="iotag")
    identb_c = nc.inline_tensor(np.eye(P, dtype=ml_dtypes.bfloat16),
                                name="identb")
    identf_c = nc.inline_tensor(np.eye(P, dtype=np.float32), name="identf")

    with tile.TileContext(nc) as tc:
        nc.gpsimd.load_library(mlp_lib)
        with tc.tile_pool(name="const", bufs=1) as cpool, \
             tc.tile_pool(name="gbuf", bufs=12) as gpool, \
             tc.tile_pool(name="ibuf", bufs=2) as ipool, \
             tc.tile_pool(name="oh", bufs=12) as ohpool, \
             tc.tile_pool(name="ohg", bufs=4) as ogpool, \
             tc.tile_pool(name="xts", bufs=2) as xpool, \
             tc.tile_pool(name="hbuf", bufs=6) as hpool, \
             tc.tile_pool(name="h2t", bufs=4) as tpool, \
             tc.tile_pool(name="sm", bufs=4) as spool, \
             tc.tile_pool(name="acc", bufs=3, space="PSUM") as apool, \
             tc.tile_pool(name="pwide", bufs=2, space="PSUM") as pwide, \
             tc.tile_pool(name="psm", bufs=1, space="PSUM") as psmall, \
             tc.tile_pool(name="pg", bufs=1, space="PSUM") as pg:

            iota_sb = cpool.tile([P, P], BF16)
            nc.sync.dma_start(out=iota_sb[:], in_=iota_c[:])
            iotag_sb = cpool.tile([P, GPG], BF16)
            nc.sync.dma_start(out=iotag_sb[:], in_=iotag_c[:])
            identb_sb = cpool.tile([P, P], BF16)
            nc.sync.dma_start(out=identb_sb[:], in_=identb_c[:])
            identf_sb = cpool.tile([P, P], FP32)
            nc.sync.dma_start(out=identf_sb[:], in_=identf_c[:])
            dstc_sb = cpool.tile([P, TOT_G], FP32)
            nc.sync.dma_start(out=dstc_sb[:], in_=dst_t[:])
            bc_sb = cpool.tile([P, NT], FP32)
            nc.sync.dma_start(out=bc_sb[:], in_=bc_t[:])
            w1_sb = cpool.tile([F, HID], BF16)
            nc.sync.dma_start(out=w1_sb[:], in_=w1_t[:])
            b1_sb = cpool.tile([HID, 1], FP32)
            nc.sync.dma_start(out=b1_sb[:], in_=b1_t[:])
            w2_sb = cpool.tile([HID, HID], BF16)
            nc.sync.dma_start(out=w2_sb[:], in_=w2_t[:])
            b2_sb = cpool.tile([HID, 1], FP32)
            nc.sync.dma_start(out=b2_sb[:], in_=b2_t[:])
            l1w_sb = cpool.tile([HID, HID], FP32)
            nc.sync.dma_start(out=l1w_sb[:], in_=l1w_t[:])
            l1b_sb = cpool.tile([HID, 1], FP32)
            nc.sync.dma_start(out=l1b_sb[:], in_=l1b_t[:])
            l2w_sb = cpool.tile([HID, NCLS], FP32)
            nc.sync.dma_start(out=l2w_sb[:], in_=l2w_t[:])
            l2b_sb = cpool.tile([GPG, NCLS], FP32)
            nc.sync.dma_start(out=l2b_sb[:], in_=l2b_t[:])

            gacc = pg.tile([GPG, HID], FP32)

            # zero-fill gather buffers once: trailing-trimmed pad slots are
            # never DMA-written, and 0 * NaN in the one-hot matmul would
            # poison PSUM if SBUF starts uninitialized
            for _mi in range(12):
                gz = gpool.tile([P, CALL_G, F], BF16, tag="gbuf")
                nc.vector.memset(ap=gz[:], constant=0.0)

            for _rep in range(rep):
                goff = 0
                ci = 0
                for tiles in chunks:
                    ntl = len(tiles)
                    cg = int(G_tb[tiles, :].sum())
                    idx_sb = ipool.tile([P, GMAXCH * 8], I16, tag="idx")
                    nc.sync.dma_start(out=idx_sb[:, :cg * 8],
                                      in_=idx_t[:, goff * 8:(goff + cg) * 8])
                    accw = []
                    for _h in range((ntl + 3) // 4):
                        acc_tile = apool.tile([P, 4 * P], FP32, tag="accw")
                        accw.append(acc_tile)

                    def acc_slice(ti):
                        return accw[ti // 4][:, (ti % 4) * P:(ti % 4 + 1) * P]

                    # one accumulation group per PSUM bank (2KB zero region):
                    # start on the bank's first matmul, stop on its last
                    BTG = [sum(int(G_tb[t, :].sum())
                               for t in tiles[h * 4:h * 4 + 4])
                           for h in range((ntl + 3) // 4)]
                    bdone = [0] * len(accw)
                    off = 0
                    for b in range(nblk):
                        nb = int(G_tb[tiles, b].sum())
                        if nb == 0:
                            continue
                        # flat (tile, j) sequence for this (chunk, block)
                        seq = []
                        for ti in range(ntl):
                            seq.extend([ti] * int(G_tb[tiles[ti], b]))
                        for c0 in range(0, nb, CALL_G):
                            ng = min(CALL_G, nb - c0)
                            gbuf = gpool.tile([P, CALL_G, F], BF16,
                                              tag="gbuf")
                            nc.gpsimd.dma_gather(
                                out_ap=gbuf[:, :ng, :],
                                in_ap=xb_t[bstart[b]:bstart[b + 1], :],
                                idxs_ap=idx_sb[:, (off + c0) * 8:
                                               (off + c0 + ng) * 8],
                                num_idxs=ng * P,
                                num_idxs_reg=ng * P,
                                elem_size=F,
                                queue_num=ci % 4,
                            )
                            ci += 1
                            for s in range(ng):
                                ti = seq[c0 + s]
                                g = goff + off + c0 + s
                                if s % 4 == 0:
                                    ohw = ohpool.tile([P, 4 * P], BF16,
                                                      tag="oh")
                                oh = ohw[:, (s % 4) * P:(s % 4 + 1) * P]
                                nc.vector.tensor_scalar(
                                    out=oh, in0=iota_sb[:],
                                    scalar1=dstc_sb[:, g:g + 1],
                                    scalar2=None,
                                    op0=mybir.AluOpType.is_equal)
                                hb = ti // 4
                                nc.tensor.matmul(
                                    out=acc_slice(ti), lhsT=gbuf[:, s, :],
                                    rhs=oh,
                                    start=(bdone[hb] == 0),
                                    stop=(bdone[hb] == BTG[hb] - 1))
                                bdone[hb] += 1
                        off += nb
                    goff += cg

                    # ---- fused phase 2 for this chunk ----
                    t0 = tiles[0]
                    w = ntl * P
                    xt_sb = xpool.tile([P, CH_T * P], FP32, tag="xt")
                    nc.sync.dma_start(out=xt_sb[:, :w],
                                      in_=xt_t[:, t0 * P:t0 * P + w])
                    hin = hpool.tile([P, CH_T * P], BF16, tag="hin")
                    for h in range((ntl + 3) // 4):
                        hw = min(4, ntl - h * 4) * P
                        nc.vector.tensor_tensor(
                            out=hin[:, h * 4 * P:h * 4 * P + hw],
                            in0=accw[h][:, :hw],
                            in1=xt_sb[:, h * 4 * P:h * 4 * P + hw],
                            op=mybir.AluOpType.add)
                    for s0 in range(0, ntl, 4):
                        sw = min(4, ntl - s0) * P
                        ps1 = pwide.tile([P, 4 * P], FP32, tag="wide")
                        nc.tensor.matmul(out=ps1[:, :sw], lhsT=w1_sb[:],
                                         rhs=hin[:, s0 * P:s0 * P + sw],
                                         start=True, stop=True)
                        h1 = hpool.tile([P, 4 * P], BF16, tag="h1")
                        nc.scalar.activation(
                            out=h1[:, :sw], in_=ps1[:, :sw],
                            func=mybir.ActivationFunctionType.Relu,
                            bias=b1_sb[:, 0:1])
                        ps2 = pwide.tile([P, 4 * P], FP32, tag="wide")
                        nc.tensor.matmul(out=ps2[:, :sw], lhsT=w2_sb[:],
                                         rhs=h1[:, :sw], start=True, stop=True)
                        h2 = hpool.tile([P, 4 * P], BF16, tag="h2")
                        nc.scalar.activation(
                            out=h2[:, :sw], in_=ps2[:, :sw],
                            func=mybir.ActivationFunctionType.Relu,
                            bias=b2_sb[:, 0:1])
                        for i in range(sw // P):
                            t = t0 + s0 + i
                            ps3 = psmall.tile([P, P], BF16, tag="ps3")
                            nc.tensor.transpose(
                                out=ps3[:], in_=h2[:, i * P:(i + 1) * P],
                                identity=identb_sb[:])
                            h2t = tpool.tile([P, P], BF16, tag="h2t")
                            nc.vector.tensor_copy(out=h2t[:], in_=ps3[:])
                            ohg = ogpool.tile([P, GPG], BF16, tag="ohg")
                            nc.vector.tensor_scalar(
                                out=ohg[:], in0=iotag_sb[:],
                                scalar1=bc_sb[:, t:t + 1], scalar2=None,
                                op0=mybir.AluOpType.is_equal)
                            nc.tensor.matmul(out=gacc[:], lhsT=ohg[:],
                                             rhs=h2t[:], start=(t == 0),
                                             stop=(t == NT - 1))

                # ---- head MLP + log_softmax ----
                g_sb = spool.tile([GPG, HID], FP32, tag="g")
                nc.scalar.copy(out=g_sb[:], in_=gacc[:])
                psg = psmall.tile([HID, GPG], FP32, tag="psh")
                nc.tensor.transpose(out=psg[:], in_=g_sb[:],
                                    identity=identf_sb[:GPG, :GPG])
                gt = spool.tile([HID, GPG], FP32, tag="gt")
                nc.vector.tensor_copy(out=gt[:], in_=psg[:])
                ps4 = psmall.tile([HID, GPG], FP32, tag="psh")
                nc.tensor.matmul(out=ps4[:], lhsT=l1w_sb[:], rhs=gt[:],
                                 start=True, stop=True)
                g1 = spool.tile([HID, GPG], FP32, tag="g1")
                nc.scalar.activation(out=g1[:], in_=ps4[:],
                                     func=mybir.ActivationFunctionType.Relu,
                                     bias=l1b_sb[:, 0:1])
                ps5 = psmall.tile([GPG, NCLS], FP32, tag="psh")
                nc.tensor.matmul(out=ps5[:], lhsT=g1[:], rhs=l2w_sb[:],
                                 start=True, stop=True)
                logits = spool.tile([GPG, NCLS], FP32, tag="lg")
                nc.vector.tensor_tensor(out=logits[:], in0=ps5[:],
                                        in1=l2b_sb[:],
                                        op=mybir.AluOpType.add)
                mx = spool.tile([GPG, 1], FP32, tag="mx")
                nc.vector.tensor_reduce(out=mx[:], in_=logits[:],
                                        axis=mybir.AxisListType.X,
                                        op=mybir.AluOpType.max)
                sh = spool.tile([GPG, NCLS], FP32, tag="sh")
                nc.vector.tensor_scalar(out=sh[:], in0=logits[:],
                                        scalar1=mx[:, 0:1], scalar2=None,
                                        op0=mybir.AluOpType.subtract)
                ex = spool.tile([GPG, NCLS], FP32, tag="ex")
                ssum = spool.tile([GPG, 1], FP32, tag="ssum")
                nc.scalar.activation(out=ex[:], in_=sh[:],
                                     func=mybir.ActivationFunctionType.Exp,
                                     accum_out=ssum[:])
                lse = spool.tile([GPG, 1], FP32, tag="lse")
                nc.scalar.activation(out=lse[:], in_=ssum[:],
                                     func=mybir.ActivationFunctionType.Ln)
                res = spool.tile([GPG, NCLS], FP32, tag="res")
                nc.vector.tensor_scalar(out=res[:], in0=sh[:],
                                        scalar1=lse[:, 0:1], scalar2=None,
                                        op0=mybir.AluOpType.subtract)
                nc.sync.dma_start(out=out_t[:], in_=res[:])

    nc.compile()
    return nc


def prep_inputs(x, edge_index, batch, conv_w1, conv_b1, conv_w2, conv_b2,
                lin1_w, lin1_b, lin2_w, lin2_b, n_graphs, blk=BLK, nblk=NBLK,
                trim=False):
    """Host-side sharding: returns (in_maps, NT, G_tb, Np, GPG)."""
    blocks = [blk] * nblk if isinstance(blk, int) else list(blk)
    nblk = len(blocks)
    bstart = np.concatenate([[0], np.cumsum(blocks)])
    n_nodes = x.shape[0]
    x = np.asarray(x, np.float32)
    batch = np.asarray(batch, np.int64)
    src = np.asarray(edge_index[0], np.int64)
    dst = np.asarray(edge_index[1], np.int64)
    gpg = n_graphs // NCORES

    bounds = np.searchsorted(batch, np.arange(0, n_graphs + 1, gpg))
    node_start = bounds[:-1]
    counts = bounds[1:] - bounds[:-1]
    NT = max(1, int(np.ceil(counts.max() / P)))
    Np = NT * P

    core = batch[dst] // gpg
    nlocal = dst - node_start[core]
    tt = nlocal // P
    dl = nlocal % P
    bb = np.searchsorted(bstart, src, side="right") - 1
    sl = src - bstart[bb]

    key = (core * NT + tt) * nblk + bb
    cnt = np.bincount(key, minlength=NCORES * NT * nblk).reshape(NCORES, NT, nblk)
    G_tb = np.ceil(cnt.max(axis=0) / P).astype(np.int64)  # [NT, nblk]
    G_tb[:, 0] = np.maximum(G_tb[:, 0], 1)
    TOT_G = int(G_tb.sum())

    # slot layout in (chunk, b, t) order — must match device emission
    chunks = _chunks(NT)
    slot_off = np.zeros((NT, nblk), np.int64)
    pos_acc = 0
    for tiles in chunks:
        for b in range(nblk):
            for t in tiles:
                slot_off[t, b] = pos_acc
                pos_acc += int(G_tb[t, b]) * P
    total_slots = TOT_G * P
    assert pos_acc == total_slots

    # bf16 x table, padded rows
    xpad = np.zeros((int(bstart[-1]), F), np.float32)
    xpad[:n_nodes] = x
    xb = xpad.astype(ml_dtypes.bfloat16)

    in_maps = []
    for c in range(NCORES):
        m = core == c
        sl_c, dl_c, tt_c, bb_c = sl[m], dl[m], tt[m], bb[m]
        order = np.lexsort((tt_c, bb_c))
        sl_c, dl_c, tt_c, bb_c = (sl_c[order], dl_c[order], tt_c[order],
                                  bb_c[order])
        # rank within bucket (edges sorted by (b, t); buckets contiguous)
        bucket = bb_c * NT + tt_c
        changes = np.concatenate([[True], bucket[1:] != bucket[:-1]])
        idx_in_run = np.arange(len(bucket)) - \
            np.maximum.accumulate(np.where(changes, np.arange(len(bucket)), 0))
        pos = slot_off[tt_c, bb_c] + idx_in_run

        SL = np.zeros(total_slots, np.int16)
        DL = np.full(total_slots, 255.0, np.float32)
        real = np.zeros(total_slots, bool)
        SL[pos] = sl_c.astype(np.int16)
        DL[pos] = dl_c.astype(np.float32)
        real[pos] = True

        # Trailing pad slots of each gather call -> idx -1 (the gather
        # ucode trims trailing negatives, skipping those DMA reads).
        goff2 = 0
        for tiles in chunks if trim else []:
            off2 = 0
            for b in range(nblk):
                nb2 = int(G_tb[tiles, b].sum())
                if nb2 == 0:
                    continue
                for c0 in range(0, nb2, CALL_G):
                    ng2 = min(CALL_G, nb2 - c0)
                    lo = (goff2 + off2 + c0) * P
                    hi = lo + ng2 * P
                    r = real[lo:hi]
                    nz = np.nonzero(r)[0]
                    tail = (nz[-1] + 1) if len(nz) else 0
                    SL[lo + tail:hi] = -1
                off2 += nb2
            goff2 += int(G_tb[tiles, :].sum())

        idx_arr = np.tile(SL.reshape(-1, 16).T, (8, 1)).astype(np.int16)
        dst_arr = DL.reshape(TOT_G, P).T.copy()

        ns, cn = node_start[c], counts[c]
        xt = np.zeros((P, Np), np.float32)
        xt[:, :cn] = x[ns:ns + cn].T
        bc = np.full(Np, 255.0, np.float32)
        bc[:cn] = (batch[ns:ns + cn] - c * gpg).astype(np.float32)
        bc = bc.reshape(NT, P).T.copy()

        in_maps.append({
            "xb": np.asarray(xb),
            "idx": idx_arr,
            "dstc": dst_arr,
            "xt": xt,
            "bc": bc,
            "w1": np.asarray(conv_w1, np.float32).astype(ml_dtypes.bfloat16),
            "b1": np.asarray(conv_b1, np.float32).reshape(HID, 1),
            "w2": np.asarray(conv_w2, np.float32).astype(ml_dtypes.bfloat16),
            "b2": np.asarray(conv_b2, np.float32).reshape(HID, 1),
            "l1w": np.asarray(lin1_w, np.float32),
            "l1b": np.asarray(lin1_b, np.float32).reshape(HID, 1),
            "l2w": np.asarray(lin2_w, np.float32),
            "l2b": np.tile(np.asarray(lin2_b, np.float32), (gpg, 1)),
        })
    return in_maps, NT, G_tb, Np, gpg


_trace = {"on": False, "last": None}


def kernel(x, edge_index, batch, conv_w1, conv_b1, conv_w2, conv_b2,
           lin1_w, lin1_b, lin2_w, lin2_b):
    n_graphs = 512
    blocks = [BLK] * NBLK
    in_maps, NT, G_tb, Np, gpg = prep_inputs(
        x, edge_index, batch, conv_w1, conv_b1, conv_w2, conv_b2,
        lin1_w, lin1_b, lin2_w, lin2_b, n_graphs, blocks, NBLK)
    nc = build_program(NT, G_tb, Np, blocks, gpg)
    res = run_bass_kernel_spmd(nc, in_maps, list(range(NCORES)),
                               trace=_trace["on"])
    _trace["last"] = res
    out = np.concatenate(
        [np.asarray(res.results[c]["out"], np.float32) for c in range(NCORES)],
        axis=0)
    return out


# revision 33
# speedup vs baseline: 1.0923x; 1.0923x over previous
"""GIN-style GNN message passing on 8 TRN2 NeuronCores.

Pipeline (per core, nodes sharded by graph id so pooling is local):
  phase 1: edge aggregation  agg[dst] += x[src]
      - node tiles processed in chunks of CH_T=8; each chunk accumulates
        directly in PSUM across all 4 src blocks (no mid-phase evacuation).
      - edges bucketed by (chunk, src block b, node tile t), padded to
        groups of 128; one dma_gather call per (chunk, b) covers all its
        groups (bf16 256B rows); segment-sum via one-hot matmul into the
        per-tile PSUM accumulator.
  phase 2 (fused per chunk): hin = bf16(x + agg); h = relu(relu(hin @ w1
      + b1) @ w2 + b2) with bf16 weights; pooled per graph via one-hot
      matmul into a single PSUM accumulator; then the small MLP head +
      log_softmax in fp32.

The bass program is identical across the 8 cores (SPMD); all data-dependent
structure (bucket sizes) is made uniform by padding to the max over cores.
"""
import numpy as np
import ml_dtypes

import concourse.bacc as bacc
import concourse.tile as tile
from concourse import mybir
from concourse.bass_utils import run_bass_kernel_spmd
from concourse.library_config import mlp as mlp_lib

P = 128
F = 128
HID = 128
NCLS = 10
NCORES = 8
CH_T = 8      # node tiles per PSUM-accumulation chunk
BLK = 25088   # uniform src block size (int16 indices)
NBLK = 4
CALL_G = 8    # max groups per dma_gather call (ring-capacity limit)

FP32 = mybir.dt.float32
BF16 = mybir.dt.bfloat16
I16 = mybir.dt.int16
I32 = mybir.dt.int32


def _n_calls(NT, G_tb, nblk):
    """Static gather-call count, matching build/prep emission order."""
    n = 0
    for tiles in _chunks(NT):
        for b in range(nblk):
            nb = int(G_tb[tiles, b].sum())
            n += (nb + CALL_G - 1) // CALL_G
    return n


def _chunks(NT):
    return [list(range(c, min(NT, c + CH_T))) for c in range(0, NT, CH_T)]


def build_program(NT, G_tb, Np, blocks, GPG, rep=1):
    """Build the SPMD bass program.

    NT: node tiles per core; G_tb: [NT, NBLK] groups per bucket; Np: NT*P;
    blocks: src block sizes; GPG: graphs per core.
    """
    nblk = len(blocks)
    bstart = [0]
    for bs in blocks:
        bstart.append(bstart[-1] + bs)
    TOT_G = int(G_tb.sum())
    chunks = _chunks(NT)
    # max groups per gather call / per chunk (uniform tile shapes)
    GMAXC = max(int(G_tb[tiles, b].sum())
                for tiles in chunks for b in range(nblk))
    GMAXCH = max(int(G_tb[tiles, :].sum()) for tiles in chunks)

    nc = bacc.Bacc("TRN2", target_bir_lowering=False, debug=False,
                   num_swdge_queues=4)

    xb_t = nc.declare_dram_parameter("xb", [bstart[-1], F], BF16, isOutput=False)
    idx_t = nc.declare_dram_parameter("idx", [P, TOT_G * 8], I16, isOutput=False)
    dst_t = nc.declare_dram_parameter("dstc", [P, TOT_G], FP32, isOutput=False)
    xt_t = nc.declare_dram_parameter("xt", [P, Np], FP32, isOutput=False)
    bc_t = nc.declare_dram_parameter("bc", [P, NT], FP32, isOutput=False)
    w1_t = nc.declare_dram_parameter("w1", [F, HID], BF16, isOutput=False)
    b1_t = nc.declare_dram_parameter("b1", [HID, 1], FP32, isOutput=False)
    w2_t = nc.declare_dram_parameter("w2", [HID, HID], BF16, isOutput=False)
    b2_t = nc.declare_dram_parameter("b2", [HID, 1], FP32, isOutput=False)
    l1w_t = nc.declare_dram_parameter("l1w", [HID, HID], FP32, isOutput=False)
    l1b_t = nc.declare_dram_parameter("l1b", [HID, 1], FP32, isOutput=False)
    l2w_t = nc.declare_dram_parameter("l2w", [HID, NCLS], FP32, isOutput=False)
    l2b_t = nc.declare_dram_parameter("l2b", [GPG, NCLS], FP32, isOutput=False)
    NCALLS = _n_calls(NT, G_tb, nblk)
    ncnt_t = nc.declare_dram_parameter("ncnt", [1, NCALLS], I32,
                                       isOutput=False)
    out_t = nc.declare_dram_parameter("out", [GPG, NCLS], FP32, isOutput=True)

    iota_c = nc.inline_tensor(
        np.tile(np.arange(P, dtype=ml_dtypes.bfloat16), (P, 1)), name="iota128")
    iotag_c = nc.inline_tensor(
        np.tile(np.arange(GPG, dtype=ml_dtypes.bfloat16), (P, 1)), name="iotag")
    identb_c = nc.inline_tensor(np.eye(P, dtype=ml_dtypes.bfloat16),
                                name="identb")
    identf_c = nc.inline_tensor(np.eye(P, dtype=np.float32), name="identf")

    with tile.TileContext(nc) as tc:
        nc.gpsimd.load_library(mlp_lib)
        with tc.tile_pool(name="const", bufs=1) as cpool, \
             tc.tile_pool(name="gbuf", bufs=12) as gpool, \
             tc.tile_pool(name="ibuf", bufs=2) as ipool, \
             tc.tile_pool(name="oh", bufs=12) as ohpool, \
             tc.tile_pool(name="ohg", bufs=4) as ogpool, \
             tc.tile_pool(name="xts", bufs=2) as xpool, \
             tc.tile_pool(name="hbuf", bufs=6) as hpool, \
             tc.tile_pool(name="h2t", bufs=4) as tpool, \
             tc.tile_pool(name="sm", bufs=4) as spool, \
             tc.tile_pool(name="acc", bufs=3, space="PSUM") as apool, \
             tc.tile_pool(name="pwide", bufs=2, space="PSUM") as pwide, \
             tc.tile_pool(name="psm", bufs=1, space="PSUM") as psmall, \
             tc.tile_pool(name="pg", bufs=1, space="PSUM") as pg:

            iota_sb = cpool.tile([P, P], BF16)
            nc.sync.dma_start(out=iota_sb[:], in_=iota_c[:])
            iotag_sb = cpool.tile([P, GPG], BF16)
            nc.sync.dma_start(out=iotag_sb[:], in_=iotag_c[:])
            identb_sb = cpool.tile([P, P], BF16)
            nc.sync.dma_start(out=identb_sb[:], in_=identb_c[:])
            identf_sb = cpool.tile([P, P], FP32)
            nc.sync.dma_start(out=identf_sb[:], in_=identf_c[:])
            dstc_sb = cpool.tile([P, TOT_G], FP32)
            nc.sync.dma_start(out=dstc_sb[:], in_=dst_t[:])
            bc_sb = cpool.tile([P, NT], FP32)
            nc.sync.dma_start(out=bc_sb[:], in_=bc_t[:])
            w1_sb = cpool.tile([F, HID], BF16)
            nc.sync.dma_start(out=w1_sb[:], in_=w1_t[:])
            b1_sb = cpool.tile([HID, 1], FP32)
            nc.sync.dma_start(out=b1_sb[:], in_=b1_t[:])
            w2_sb = cpool.tile([HID, HID], BF16)
            nc.sync.dma_start(out=w2_sb[:], in_=w2_t[:])
            b2_sb = cpool.tile([HID, 1], FP32)
            nc.sync.dma_start(out=b2_sb[:], in_=b2_t[:])
            l1w_sb = cpool.tile([HID, HID], FP32)
            nc.sync.dma_start(out=l1w_sb[:], in_=l1w_t[:])
            l1b_sb = cpool.tile([HID, 1], FP32)
            nc.sync.dma_start(out=l1b_sb[:], in_=l1b_t[:])
            l2w_sb = cpool.tile([HID, NCLS], FP32)
            nc.sync.dma_start(out=l2w_sb[:], in_=l2w_t[:])
            l2b_sb = cpool.tile([GPG, NCLS], FP32)
            nc.sync.dma_start(out=l2b_sb[:], in_=l2b_t[:])
            ncnt_sb = cpool.tile([1, NCALLS], I32)
            nc.sync.dma_start(out=ncnt_sb[:], in_=ncnt_t[:])
            # per-core real gather count, loaded into a Pool register per
            # call (trailing pad slots carry idx -1 and are DMA-skipped)
            nreg = nc.gpsimd.alloc_register("gather_cnt")

            gacc = pg.tile([GPG, HID], FP32)

            # zero-fill gather buffers once: trailing-trimmed pad slots are
            # never DMA-written, and 0 * NaN in the one-hot matmul would
            # poison PSUM if SBUF starts uninitialized
            for _mi in range(12):
                gz = gpool.tile([P, CALL_G, F], BF16, tag="gbuf")
                nc.vector.memset(ap=gz[:], constant=0.0)

            for _rep in range(rep):
                goff = 0
                ci = 0
                for tiles in chunks:
                    ntl = len(tiles)
                    cg = int(G_tb[tiles, :].sum())
                    idx_sb = ipool.tile([P, GMAXCH * 8], I16, tag="idx")
                    nc.sync.dma_start(out=idx_sb[:, :cg * 8],
                                      in_=idx_t[:, goff * 8:(goff + cg) * 8])
                    accw = []
                    for _h in range((ntl + 3) // 4):
                        acc_tile = apool.tile([P, 4 * P], FP32, tag="accw")
                        accw.append(acc_tile)

                    def acc_slice(ti):
                        return accw[ti // 4][:, (ti % 4) * P:(ti % 4 + 1) * P]

                    # one accumulation group per PSUM bank (2KB zero region):
                    # start on the bank's first matmul, stop on its last
                    BTG = [sum(int(G_tb[t, :].sum())
                               for t in tiles[h * 4:h * 4 + 4])
                           for h in range((ntl + 3) // 4)]
                    bdone = [0] * len(accw)
                    off = 0
                    for b in range(nblk):
                        nb = int(G_tb[tiles, b].sum())
                        if nb == 0:
                            continue
                        # flat (tile, j) sequence for this (chunk, block)
                        seq = []
                        for ti in range(ntl):
                            seq.extend([ti] * int(G_tb[tiles[ti], b]))
                        for c0 in range(0, nb, CALL_G):
                            ng = min(CALL_G, nb - c0)
                            gbuf = gpool.tile([P, CALL_G, F], BF16,
                                              tag="gbuf")
                            nc.gpsimd.reg_load(nreg,
                                               ncnt_sb[0:1, ci:ci + 1])
                            nc.gpsimd.dma_gather(
                                out_ap=gbuf[:, :ng, :],
                                in_ap=xb_t[bstart[b]:bstart[b + 1], :],
                                idxs_ap=idx_sb[:, (off + c0) * 8:
                                               (off + c0 + ng) * 8],
                                num_idxs=ng * P,
                                num_idxs_reg=nreg,
                                elem_size=F,
                                queue_num=ci % 4,
                            )
                            ci += 1
                            for s in range(ng):
                                ti = seq[c0 + s]
                                g = goff + off + c0 + s
                                if s % 4 == 0:
                                    ohw = ohpool.tile([P, 4 * P], BF16,
                                                      tag="oh")
                                oh = ohw[:, (s % 4) * P:(s % 4 + 1) * P]
                                nc.vector.tensor_scalar(
                                    out=oh, in0=iota_sb[:],
                                    scalar1=dstc_sb[:, g:g + 1],
                                    scalar2=None,
                                    op0=mybir.AluOpType.is_equal)
                                hb = ti // 4
                                nc.tensor.matmul(
                                    out=acc_slice(ti), lhsT=gbuf[:, s, :],
                                    rhs=oh,
                                    start=(bdone[hb] == 0),
                                    stop=(bdone[hb] == BTG[hb] - 1))
                                bdone[hb] += 1
                        off += nb
                    goff += cg

                    # ---- fused phase 2 for this chunk ----
                    t0 = tiles[0]
                    w = ntl * P
                    xt_sb = xpool.tile([P, CH_T * P], FP32, tag="xt")
                    nc.sync.dma_start(out=xt_sb[:, :w],
                                      in_=xt_t[:, t0 * P:t0 * P + w])
                    hin = hpool.tile([P, CH_T * P], BF16, tag="hin")
                    for h in range((ntl + 3) // 4):
                        hw = min(4, ntl - h * 4) * P
                        nc.vector.tensor_tensor(
                            out=hin[:, h * 4 * P:h * 4 * P + hw],
                            in0=accw[h][:, :hw],
                            in1=xt_sb[:, h * 4 * P:h * 4 * P + hw],
                            op=mybir.AluOpType.add)
                    for s0 in range(0, ntl, 4):
                        sw = min(4, ntl - s0) * P
                        ps1 = pwide.tile([P, 4 * P], FP32, tag="wide")
                        nc.tensor.matmul(out=ps1[:, :sw], lhsT=w1_sb[:],
                                         rhs=hin[:, s0 * P:s0 * P + sw],
                                         start=True, stop=True)
                        h1 = hpool.tile([P, 4 * P], BF16, tag="h1")
                        nc.scalar.activation(
                            out=h1[:, :sw], in_=ps1[:, :sw],
                            func=mybir.ActivationFunctionType.Relu,
                            bias=b1_sb[:, 0:1])
                        ps2 = pwide.tile([P, 4 * P], FP32, tag="wide")
                        nc.tensor.matmul(out=ps2[:, :sw], lhsT=w2_sb[:],
                                         rhs=h1[:, :sw], start=True, stop=True)
                        h2 = hpool.tile([P, 4 * P], BF16, tag="h2")
                        nc.scalar.activation(
                            out=h2[:, :sw], in_=ps2[:, :sw],
                            func=mybir.ActivationFunctionType.Relu,
                            bias=b2_sb[:, 0:1])
                        for i in range(sw // P):
                            t = t0 + s0 + i
                            ps3 = psmall.tile([P, P], BF16, tag="ps3")
                            nc.tensor.transpose(
                                out=ps3[:], in_=h2[:, i * P:(i + 1) * P],
                                identity=identb_sb[:])
                            h2t = tpool.tile([P, P], BF16, tag="h2t")
                            nc.vector.tensor_copy(out=h2t[:], in_=ps3[:])
                            ohg = ogpool.tile([P, GPG], BF16, tag="ohg")
                            nc.vector.tensor_scalar(
                                out=ohg[:], in0=iotag_sb[:],
                                scalar1=bc_sb[:, t:t + 1], scalar2=None,
                                op0=mybir.AluOpType.is_equal)
                            nc.tensor.matmul(out=gacc[:], lhsT=ohg[:],
                                             rhs=h2t[:], start=(t == 0),
                                             stop=(t == NT - 1))

                # ---- head MLP + log_softmax ----
                g_sb = spool.tile([GPG, HID], FP32, tag="g")
                nc.scalar.copy(out=g_sb[:], in_=gacc[:])
                psg = psmall.tile([HID, GPG], FP32, tag="psh")
                nc.tensor.transpose(out=psg[:], in_=g_sb[:],
                                    identity=identf_sb[:GPG, :GPG])
                gt = spool.tile([HID, GPG], FP32, tag="gt")
                nc.vector.tensor_copy(out=gt[:], in_=psg[:])
                ps4 = psmall.tile([HID, GPG], FP32, tag="psh")
                nc.tensor.matmul(out=ps4[:], lhsT=l1w_sb[:], rhs=gt[:],
                                 start=True, stop=True)
                g1 = spool.tile([HID, GPG], FP32, tag="g1")
                nc.scalar.activation(out=g1[:], in_=ps4[:],
                                     func=mybir.ActivationFunctionType.Relu,
                                     bias=l1b_sb[:, 0:1])
                ps5 = psmall.tile([GPG, NCLS], FP32, tag="psh")
                nc.tensor.matmul(out=ps5[:], lhsT=g1[:], rhs=l2w_sb[:],
                                 start=True, stop=True)
                logits = spool.tile([GPG, NCLS], FP32, tag="lg")
                nc.vector.tensor_tensor(out=logits[:], in0=ps5[:],
                                        in1=l2b_sb[:],
                                        op=mybir.AluOpType.add)
                mx = spool.tile([GPG, 1], FP32, tag="mx")
                nc.vector.tensor_reduce(out=mx[:], in_=logits[:],
                                        axis=mybir.AxisListType.X,
                                        op=mybir.AluOpType.max)
                sh = spool.tile([GPG, NCLS], FP32, tag="sh")
                nc.vector.tensor_scalar(out=sh[:], in0=logits[:],
                                        scalar1=mx[:, 0:1], scalar2=None,
                                        op0=mybir.AluOpType.subtract)
                ex = spool.tile([GPG, NCLS], FP32, tag="ex")
                ssum = spool.tile([GPG, 1], FP32, tag="ssum")
                nc.scalar.activation(out=ex[:], in_=sh[:],
                                     func=mybir.ActivationFunctionType.Exp,
                                     accum_out=ssum[:])
                lse = spool.tile([GPG, 1], FP32, tag="lse")
                nc.scalar.activation(out=lse[:], in_=ssum[:],
                                     func=mybir.ActivationFunctionType.Ln)
                res = spool.tile([GPG, NCLS], FP32, tag="res")
                nc.vector.tensor_scalar(out=res[:], in0=sh[:],
                                        scalar1=lse[:, 0:1], scalar2=None,
                                        op0=mybir.AluOpType.subtract)
                nc.sync.dma_start(out=out_t[:], in_=res[:])

    nc.compile()
    return nc


def prep_inputs(x, edge_index, batch, conv_w1, conv_b1, conv_w2, conv_b2,
                lin1_w, lin1_b, lin2_w, lin2_b, n_graphs, blk=BLK, nblk=NBLK,
                trim=True):
    """Host-side sharding: returns (in_maps, NT, G_tb, Np, GPG)."""
    blocks = [blk] * nblk if isinstance(blk, int) else list(blk)
    nblk = len(blocks)
    bstart = np.concatenate([[0], np.cumsum(blocks)])
    n_nodes = x.shape[0]
    x = np.asarray(x, np.float32)
    batch = np.asarray(batch, np.int64)
    src = np.asarray(edge_index[0], np.int64)
    dst = np.asarray(edge_index[1], np.int64)
    gpg = n_graphs // NCORES

    bounds = np.searchsorted(batch, np.arange(0, n_graphs + 1, gpg))
    node_start = bounds[:-1]
    counts = bounds[1:] - bounds[:-1]
    NT = max(1, int(np.ceil(counts.max() / P)))
    Np = NT * P

    core = batch[dst] // gpg
    nlocal = dst - node_start[core]
    tt = nlocal // P
    dl = nlocal % P
    bb = np.searchsorted(bstart, src, side="right") - 1
    sl = src - bstart[bb]

    key = (core * NT + tt) * nblk + bb
    cnt = np.bincount(key, minlength=NCORES * NT * nblk).reshape(NCORES, NT, nblk)
    G_tb = np.ceil(cnt.max(axis=0) / P).astype(np.int64)  # [NT, nblk]
    G_tb[:, 0] = np.maximum(G_tb[:, 0], 1)
    TOT_G = int(G_tb.sum())

    # slot layout in (chunk, b, t) order — must match device emission
    chunks = _chunks(NT)
    slot_off = np.zeros((NT, nblk), np.int64)
    pos_acc = 0
    for tiles in chunks:
        for b in range(nblk):
            for t in tiles:
                slot_off[t, b] = pos_acc
                pos_acc += int(G_tb[t, b]) * P
    total_slots = TOT_G * P
    assert pos_acc == total_slots

    # bf16 x table, padded rows
    xpad = np.zeros((int(bstart[-1]), F), np.float32)
    xpad[:n_nodes] = x
    xb = xpad.astype(ml_dtypes.bfloat16)

    in_maps = []
    for c in range(NCORES):
        m = core == c
        sl_c, dl_c, tt_c, bb_c = sl[m], dl[m], tt[m], bb[m]
        order = np.lexsort((tt_c, bb_c))
        sl_c, dl_c, tt_c, bb_c = (sl_c[order], dl_c[order], tt_c[order],
                                  bb_c[order])
        # rank within bucket (edges sorted by (b, t); buckets contiguous)
        bucket = bb_c * NT + tt_c
        changes = np.concatenate([[True], bucket[1:] != bucket[:-1]])
        idx_in_run = np.arange(len(bucket)) - \
            np.maximum.accumulate(np.where(changes, np.arange(len(bucket)), 0))
        pos = slot_off[tt_c, bb_c] + idx_in_run

        SL = np.zeros(total_slots, np.int16)
        DL = np.full(total_slots, 255.0, np.float32)
        real = np.zeros(total_slots, bool)
        SL[pos] = sl_c.astype(np.int16)
        DL[pos] = dl_c.astype(np.float32)
        real[pos] = True

        # Trailing pad slots of each gather call -> idx -1 (the gather
        # ucode trims trailing negatives, skipping those DMA reads).
        # ncnt[call] = per-core count of non-negative idxs, loaded into the
        # gather's num_idxs_reg at runtime (the ucode contract).
        ncnt = []
        goff2 = 0
        for tiles in chunks:
            off2 = 0
            for b in range(nblk):
                nb2 = int(G_tb[tiles, b].sum())
                if nb2 == 0:
                    continue
                for c0 in range(0, nb2, CALL_G):
                    ng2 = min(CALL_G, nb2 - c0)
                    lo = (goff2 + off2 + c0) * P
                    hi = lo + ng2 * P
                    if trim:
                        r = real[lo:hi]
                        nz = np.nonzero(r)[0]
                        tail = int(nz[-1] + 1) if len(nz) else 0
                        # keep >=16 live idxs (one stripe): an all-negative
                        # call is degenerate in the gather ucode
                        tail = max(tail, 16)
                        SL[lo + tail:hi] = -1
                        ncnt.append(tail)
                    else:
                        ncnt.append(ng2 * P)
                off2 += nb2
            goff2 += int(G_tb[tiles, :].sum())
        ncnt = np.asarray(ncnt, np.int32).reshape(1, -1)

        idx_arr = np.tile(SL.reshape(-1, 16).T, (8, 1)).astype(np.int16)
        dst_arr = DL.reshape(TOT_G, P).T.copy()

        ns, cn = node_start[c], counts[c]
        xt = np.zeros((P, Np), np.float32)
        xt[:, :cn] = x[ns:ns + cn].T
        bc = np.full(Np, 255.0, np.float32)
        bc[:cn] = (batch[ns:ns + cn] - c * gpg).astype(np.float32)
        bc = bc.reshape(NT, P).T.copy()

        in_maps.append({
            "xb": np.asarray(xb),
            "idx": idx_arr,
            "dstc": dst_arr,
            "xt": xt,
            "bc": bc,
            "w1": np.asarray(conv_w1, np.float32).astype(ml_dtypes.bfloat16),
            "b1": np.asarray(conv_b1, np.float32).reshape(HID, 1),
            "w2": np.asarray(conv_w2, np.float32).astype(ml_dtypes.bfloat16),
            "b2": np.asarray(conv_b2, np.float32).reshape(HID, 1),
            "l1w": np.asarray(lin1_w, np.float32),
            "l1b": np.asarray(lin1_b, np.float32).reshape(HID, 1),
            "l2w": np.asarray(lin2_w, np.float32),
            "l2b": np.tile(np.asarray(lin2_b, np.float32), (gpg, 1)),
            "ncnt": ncnt,
        })
    return in_maps, NT, G_tb, Np, gpg


_trace = {"on": False, "last": None}


def kernel(x, edge_index, batch, conv_w1, conv_b1, conv_w2, conv_b2,
           lin1_w, lin1_b, lin2_w, lin2_b):
    n_graphs = 512
    blocks = [BLK] * NBLK
    in_maps, NT, G_tb, Np, gpg = prep_inputs(
        x, edge_index, batch, conv_w1, conv_b1, conv_w2, conv_b2,
        lin1_w, lin1_b, lin2_w, lin2_b, n_graphs, blocks, NBLK)
    nc = build_program(NT, G_tb, Np, blocks, gpg)
    res = run_bass_kernel_spmd(nc, in_maps, list(range(NCORES)),
                               trace=_trace["on"])
    _trace["last"] = res
    out = np.concatenate(
        [np.asarray(res.results[c]["out"], np.float32) for c in range(NCORES)],
        axis=0)
    return out
